# revision 1
# baseline (speedup 1.0000x reference)
"""Causal attention (B=4, S=2048, D=1024) on 8 Trainium2 NeuronCores.

Sharding: core c = (batch b = c//2, half h = c%2). Each core computes the
full attention output for 1024 query rows (rows [h*1024, (h+1)*1024) of
batch b), against the full 2048-key sequence of that batch.

Per-core kernel (SPMD, identical program, per-core data):
  Phase A: project Q^T, K^T (fp32, SBUF-resident) and V (bf16,
           SBUF-resident) from host-pre-transposed x^T and W^T inputs.
  Phase B: transposed-scores flash attention: S^T = K @ Q^T (fp32r
           matmuls), additive mask bias from the actual mask input,
           exp on ScalarE (no max subtraction; scores are ~N(0,1) by
           construction so exp is safe), P in bf16, O = P @ V and
           l = P^T-column sums accumulated in PSUM across all 16
           key blocks, then normalize O by 1/l and DMA out.
"""

import sys

sys.path.insert(0, "/opt/trn_rl_repo")

import numpy as np

import concourse.bass as bass
import concourse.mybir as mybir
from concourse import tile
from concourse.bass_utils import run_bass_kernel_spmd

F32 = mybir.dt.float32
F32R = mybir.dt.float32r
BF16 = mybir.dt.bfloat16
AF = mybir.ActivationFunctionType

B, S, D = 4, 2048, 1024
NQ = 1024          # query rows per core
NKB = 16           # key blocks of 128
NQC = 4            # query column chunks per core
QW = 256           # query width of one score tile
NMC = 8            # d_model chunks of 128 (contraction)
NDC = 8            # d_k chunks of 128
NEG = -1.0e6
SCALE = 1.0 / 32.0  # 1/sqrt(D_K)


def _set_dims(s, d, nq, qw):
    global S, D, NQ, NKB, NQC, QW, NMC, NDC, SCALE
    S, D, NQ, QW = s, d, nq, qw
    NKB = S // 128
    NQC = NQ // QW
    NMC = D // 128
    NDC = D // 128
    SCALE = 1.0 / float(np.sqrt(D))


def _build_nc():
    nc = bass.Bass()
    xqT = nc.declare_dram_parameter("xqT", [D, NQ], F32R, isOutput=False)
    xkvT = nc.declare_dram_parameter("xkvT", [D, S], F32R, isOutput=False)
    wqT = nc.declare_dram_parameter("wqT", [D, D], F32R, isOutput=False)
    wkT = nc.declare_dram_parameter("wkT", [D, D], F32R, isOutput=False)
    wvT = nc.declare_dram_parameter("wvT", [D, D], F32R, isOutput=False)
    biasT = nc.declare_dram_parameter("biasT", [NQC, NKB, 128, QW], F32, isOutput=False)
    out = nc.declare_dram_parameter("out", [NQ, D], F32, isOutput=True)

    with tile.TileContext(nc) as tc:
        with tc.tile_pool(name="res", bufs=1) as res, \
             tc.tile_pool(name="rawp", bufs=3) as rawp, \
             tc.tile_pool(name="psp", bufs=1, space="PSUM") as psp:
            # Resident: Q^T [p=dk, dc, q]; K^T [p=dk, dc, k]; V [p=k, kb, dv].
            qt_sb = res.tile([128, NDC * NQ], F32R, name="qt_sb")
            kt_sb = res.tile([128, NDC * S], F32R, name="kt_sb")
            v_sb = res.tile([128, NKB * D], BF16, name="v_sb")
            ones = res.tile([128, 1], BF16, name="ones")
            nc.vector.memset(ones[:], 1.0)

            def stage(dst_ap, dram_ap, ncols, nchunks, name):
                # DMA dram chunks into a raw tile, then one DVE copy -> dst.
                # Keeps every matmul input produced by DVE (1-wait rule).
                raw = rawp.tile([128, ncols * nchunks], F32R, name=name, tag="raw")
                for i in range(nchunks):
                    nc.sync.dma_start(
                        raw[:, i * ncols:(i + 1) * ncols], dram_ap(i)
                    )
                nc.vector.tensor_copy(dst_ap, raw[:])

            # ---------------- Phase A1: Q^T = Wq @ x_q^T ----------------
            with (
                tc.tile_pool(name="xqp", bufs=1) as xqp,
                tc.tile_pool(name="wqp", bufs=2) as wqp,
            ):
                xq_sb = xqp.tile([128, NMC * NQ], F32R, name="xq_sb")
                for mc in range(NMC):
                    stage(
                        xq_sb[:, mc * NQ:(mc + 1) * NQ],
                        lambda i, mc=mc: xqT[mc * 128:(mc + 1) * 128, :],
                        NQ, 1, "rxq",
                    )
                for dc in range(NDC):
                    wsl = wqp.tile([128, NMC * 128], F32R, name="wsl", tag="wsl")
                    stage(
                        wsl[:],
                        lambda i, dc=dc: wqT[i * 128:(i + 1) * 128, dc * 128:(dc + 1) * 128],
                        128, NMC, "rwq",
                    )
                    W1 = min(512, NQ)
                    for q2 in range(NQ // W1):
                        ps = psp.tile([128, 512], F32, name="psq", tag="st", bufs=2)[:, :W1]
                        for mc in range(NMC):
                            nc.tensor.matmul(
                                ps[:],
                                lhsT=wsl[:, mc * 128:(mc + 1) * 128],
                                rhs=xq_sb[:, mc * NQ + q2 * W1: mc * NQ + q2 * W1 + W1],
                                start=(mc == 0),
                                stop=(mc == NMC - 1),
                            )
                        nc.vector.tensor_copy(
                            qt_sb[:, dc * NQ + q2 * W1: dc * NQ + q2 * W1 + W1], ps[:]
                        )

            # ---------------- Phase A2: K^T = Wk @ x_kv^T ----------------
            with (
                tc.tile_pool(name="wkp", bufs=1) as wkp,
                tc.tile_pool(name="xcp", bufs=2) as xcp,
            ):
                wk_sb = wkp.tile([128, NMC * D], F32R, name="wk_sb")
                for mc in range(NMC):
                    stage(
                        wk_sb[:, mc * D:(mc + 1) * D],
                        lambda i, mc=mc: wkT[mc * 128:(mc + 1) * 128, :],
                        D, 1, "rwk",
                    )
                for _pi in range(2):
                    _pt = psp.tile([128, 512], F32, name=f"prime{_pi}", tag="st", bufs=2)
                    nc.vector.memset(_pt[:], 0.0)
                for kc in range(S // 256):  # 256-wide key column chunks
                    xcol = xcp.tile([128, NMC * 256], F32R, name="xcol", tag="xcol")
                    stage(
                        xcol[:],
                        lambda i, kc=kc: xkvT[i * 128:(i + 1) * 128, kc * 256:(kc + 1) * 256],
                        256, NMC, "rxc",
                    )
                    for dc in range(NDC):
                        ps = psp.tile([128, 512], F32, name="psk", tag="st", bufs=2)[:, :256]
                        for mc in range(NMC):
                            nc.tensor.matmul(
                                ps[:],
                                lhsT=wk_sb[:, mc * D + dc * 128: mc * D + dc * 128 + 128],
                                rhs=xcol[:, mc * 256:(mc + 1) * 256],
                                start=(mc == 0),
                                stop=(mc == NMC - 1),
                            )
                        nc.vector.tensor_copy(
                            kt_sb[:, dc * S + kc * 256: dc * S + kc * 256 + 256], ps[:]
                        )

            # ---------------- Phase A3: V = x_kv @ Wv^T (bf16) ----------------
            with (
                tc.tile_pool(name="wvp", bufs=1) as wvp,
                tc.tile_pool(name="xcp2", bufs=2) as xcp,
            ):
                wv_sb = wvp.tile([128, NMC * D], F32R, name="wv_sb")
                for mc in range(NMC):
                    stage(
                        wv_sb[:, mc * D:(mc + 1) * D],
                        lambda i, mc=mc: wvT[mc * 128:(mc + 1) * 128, :],
                        D, 1, "rwv",
                    )
                for _pi in range(2):
                    _pt = psp.tile([128, 512], F32, name=f"prime{_pi}", tag="st", bufs=2)
                    nc.vector.memset(_pt[:], 0.0)
                for kc in range(S // 256):
                    xcol = xcp.tile([128, NMC * 256], F32R, name="xcol2", tag="xcol2")
                    stage(
                        xcol[:],
                        lambda i, kc=kc: xkvT[i * 128:(i + 1) * 128, kc * 256:(kc + 1) * 256],
                        256, NMC, "rxc2",
                    )
                    DV = min(512, D)
                    for kbl in range(2):
                        kb = kc * 2 + kbl
                        for dvc in range(D // DV):
                            ps = psp.tile([128, 512], F32, name="psv", tag="st", bufs=2)[:, :DV]
                            for mc in range(NMC):
                                nc.tensor.matmul(
                                    ps[:],
                                    lhsT=xcol[:, mc * 256 + kbl * 128: mc * 256 + kbl * 128 + 128],
                                    rhs=wv_sb[:, mc * D + dvc * DV: mc * D + dvc * DV + DV],
                                    start=(mc == 0),
                                    stop=(mc == NMC - 1),
                                )
                            nc.vector.tensor_copy(
                                v_sb[:, kb * D + dvc * DV: kb * D + dvc * DV + DV],
                                ps[:],
                            )

            # ---------------- Phase B: attention ----------------
            with (
                tc.tile_pool(name="bp", bufs=3) as bp,
                tc.tile_pool(name="sfp", bufs=3) as sfp,
                tc.tile_pool(name="pap", bufs=3) as pap,
                tc.tile_pool(name="pep", bufs=3) as pep,
                tc.tile_pool(name="otp", bufs=2) as otp,
                tc.tile_pool(name="rcp", bufs=2) as rcp,
            ):
                for _pi in range(2):
                    _pt = psp.tile([128, 512], F32, name=f"prime{_pi}", tag="st", bufs=2)
                    nc.vector.memset(_pt[:], 0.0)
                NQB = QW // 128
                DV = min(512, D)
                NDV = D // DV
                for qc in range(NQC):
                    o_ps = [
                        psp.tile([128, DV], F32, name=f"o_ps{i}", tag=f"o{i}")
                        for i in range(NQB * NDV)
                    ]
                    l_ps = [
                        psp.tile([128, 1], F32, name=f"l_ps{qb}", tag=f"l{qb}")
                        for qb in range(NQB)
                    ]
                    for j in range(NKB):
                        st = psp.tile([128, 512], F32, name="st", tag="st", bufs=2)[:, :QW]
                        for dc in range(NDC):
                            nc.tensor.matmul(
                                st[:],
                                lhsT=kt_sb[:, dc * S + j * 128: dc * S + j * 128 + 128],
                                rhs=qt_sb[:, dc * NQ + qc * QW: dc * NQ + qc * QW + QW],
                                start=(dc == 0),
                                stop=(dc == NDC - 1),
                            )
                        bt = bp.tile([128, QW], F32, name="bt", tag="bt")
                        nc.sync.dma_start(bt[:], biasT[qc, j])
                        sf = sfp.tile([128, QW], F32, name="sf", tag="sf")
                        nc.vector.tensor_add(sf[:], st[:], bt[:])
                        pa = pap.tile([128, QW], BF16, name="pa", tag="pa")
                        nc.scalar.activation(pa[:], sf[:], AF.Exp, scale=SCALE)
                        pe = pep.tile([128, QW], BF16, name="pe", tag="pe")
                        nc.vector.tensor_copy(pe[:], pa[:])
                        for qb in range(NQB):
                            nc.tensor.matmul(
                                l_ps[qb][:],
                                lhsT=pe[:, qb * 128:(qb + 1) * 128],
                                rhs=ones[:],
                                start=(j == 0),
                                stop=(j == NKB - 1),
                            )
                            for dvc in range(NDV):
                                nc.tensor.matmul(
                                    o_ps[qb * NDV + dvc][:],
                                    lhsT=pe[:, qb * 128:(qb + 1) * 128],
                                    rhs=v_sb[:, j * D + dvc * DV: j * D + dvc * DV + DV],
                                    start=(j == 0),
                                    stop=(j == NKB - 1),
                                )
                    for qb in range(NQB):
                        rc = rcp.tile([128, 1], F32, name="rc", tag="rc")
                        nc.vector.reciprocal(rc[:], l_ps[qb][:])
                        for dvc in range(NDV):
                            ot = otp.tile([128, DV], F32, name="ot", tag="ot")
                            nc.vector.tensor_scalar_mul(
                                ot[:], o_ps[qb * NDV + dvc][:], rc[:]
                            )
                            nc.sync.dma_start(
                                out[
                                    qc * QW + qb * 128: qc * QW + qb * 128 + 128,
                                    dvc * DV: dvc * DV + DV,
                                ],
                                ot[:],
                            )
    _elide_transitive_waits(nc)
    return nc


def _elide_transitive_waits(nc):
    """Drop semaphore waits already implied transitively.

    Hardware matmul (fused LDWEIGHTS) and DMA instruction encodings accept
    only ONE sync wait.  Tile's wait assignment is per-proc minimal but NOT
    transitive, so phase boundaries emit multi-wait matmuls/DMAs.  This pass
    walks the scheduled program (list order is a valid linearization),
    maintains a transitive vector clock per proc (engines and DMA queues are
    each FIFO), and removes waits that are (a) on the instruction's own proc
    (FIFO completion order), or (b) already implied by an earlier retained
    wait's transitive closure.
    """
    import re
    _proc_re = re.compile(r"^(PE|DVE|ACT|Act|Activation|SP|Pool|POOL|DMAHW\d+|DMASW\d+)_")

    def _is_proc_sem(name):
        return bool(_proc_re.match(name or ""))

    hist = {}      # sem id -> list of (tick, snapshot dict)
    state = {}     # proc key -> dict(sem id -> observed tick)
    tickc = {}     # sem id -> cumulative tick

    def snap_at(sem, t):
        h = hist.get(sem)
        if not h:
            return None
        lo, hi, best = 0, len(h) - 1, None
        while lo <= hi:
            mid = (lo + hi) // 2
            if h[mid][0] <= t:
                best = h[mid][1]
                lo = mid + 1
            else:
                hi = mid - 1
        return best

    splits = []
    for blk in nc.m.functions[0].blocks:
        for idx, i in enumerate(blk.instructions):
            si = i.sync_info
            if si is None:
                continue
            ups = [u for u in si.on_update if _is_proc_sem(u.ant_name)]
            own = ups[0].id if ups else ("eng", str(i.engine))
            v = state.setdefault(own, {})
            keep = []
            for w in list(si.on_wait):
                if (
                    w.wait_mode != "sem-ge-imm"
                    or w.wait_reg is not None
                    or not _is_proc_sem(w.ant_name)
                ):
                    keep.append(w)
                    continue
                # Same-proc elision is ONLY safe for PE matmuls: the PE
                # completes matmuls strictly in order (pc-monotone ends), so
                # a PE-self completion wait is redundant.  Other engines have
                # deep pipelines where same-engine WAR/WAW needs the wait.
                pe_self = (
                    w.id == own
                    and type(i).__name__ in ("InstMatmult", "InstLdweights")
                    and w.ant_name.startswith("PE")
                )
                if pe_self or v.get(w.id, 0) >= w.wait_value:
                    continue  # implied: PE FIFO or transitive closure
                keep.append(w)
                v[w.id] = max(v.get(w.id, 0), w.wait_value)
                s = snap_at(w.id, w.wait_value)
                if s:
                    for k2, t2 in s.items():
                        if v.get(k2, 0) < t2:
                            v[k2] = t2
            if len(keep) > 1 and all(_is_proc_sem(w.ant_name) for w in keep):
                # Hardware instruction encodings here accept at most one
                # sync wait: hoist all waits onto standalone sequencer
                # event-semaphore wait ops inserted just before.
                for k, w in enumerate(keep):
                    splits.append(
                        (blk, idx, mybir.InstEventSemaphore(
                            name=f"{i.name}-w{k}",
                            engine=i.engine,
                            sync_info=mybir.SyncInfo(on_wait=[w], on_update=[]),
                        ))
                    )
                keep = []
            if len(keep) != len(si.on_wait):
                si.on_wait = keep
                i.sync_info = si
            for u in ups:
                inc = u.update_value if u.update_mode in ("sem-inc", "sem-add-imm") else 0
                t = tickc.get(u.id, 0) + (inc or 0)
                tickc[u.id] = t
                snapshot = dict(v)
                snapshot[u.id] = t
                hist.setdefault(u.id, []).append((t, snapshot))
            nm = type(i).__name__
            if nm in ("InstMatmult", "InstDMACopy", "InstTensorCopy",
                      "InstTensorTensor", "InstActivation", "InstMemset",
                      "InstTensorScalarPtr", "InstReciprocal", "InstLdweights"):
                assert len(i.sync_info.on_wait) <= 1, (
                    i.name, nm,
                    [(w.ant_name, w.wait_value) for w in i.sync_info.on_wait],
                )
    by_blk = {}
    for blk, idx, inst in splits:
        by_blk.setdefault(id(blk), (blk, []))[1].append((idx, inst))
    for blk, items in by_blk.values():
        for idx, inst in sorted(items, key=lambda t: -t[0]):
            nc.register_instruction(inst)
            blk.instructions.insert(idx, inst)


_CACHE = {}


def _get_nc():
    if "nc" not in _CACHE:
        _CACHE["nc"] = _build_nc()
    return _CACHE["nc"]


def make_in_maps(x, mask, Wq, Wk, Wv):
    x = np.asarray(x, dtype=np.float32)
    mask = np.asarray(mask)
    wqT = np.ascontiguousarray(np.asarray(Wq, np.float32).T)
    wkT = np.ascontiguousarray(np.asarray(Wk, np.float32).T)
    wvT = np.ascontiguousarray(np.asarray(Wv, np.float32).T)
    in_maps = []
    for c in range(8):
        b, h = divmod(c, 2)
        xb = x[b]
        xqT = np.ascontiguousarray(xb[h * NQ:(h + 1) * NQ].T)
        xkvT = np.ascontiguousarray(xb.T)
        mb = mask[b, h * NQ:(h + 1) * NQ, :]  # [1024 q, 2048 k]
        mt = mb.T.reshape(NKB, 128, NQC, QW).transpose(2, 0, 1, 3)
        bias = np.where(mt, np.float32(0.0), np.float32(NEG))
        in_maps.append(
            dict(
                xqT=xqT,
                xkvT=xkvT,
                wqT=wqT,
                wkT=wkT,
                wvT=wvT,
                biasT=np.ascontiguousarray(bias),
            )
        )
    return in_maps


def assemble(results):
    out = np.empty((B, S, D), np.float32)
    for c in range(8):
        b, h = divmod(c, 2)
        out[b, h * NQ:(h + 1) * NQ] = results[c]["out"]
    return out


def kernel(x, mask, Wq, Wk, Wv):
    nc = _get_nc()
    in_maps = make_in_maps(x, mask, Wq, Wk, Wv)
    res = run_bass_kernel_spmd(nc, in_maps, list(range(8)))
    return assemble(res.results)



# revision 16
# speedup vs baseline: 787.3108x; 787.3108x over previous
"""Causal attention (B=4, S=2048, D=1024) on 8 Trainium2 NeuronCores.

Sharding (key-parallel, causality-aware, load-balanced): core c =
(batch b = c//2, fold f = c%2). Core f owns the 8 interleaved key blocks
{f, f+2, ..., f+14} (128 keys each) of its batch and computes, for ALL
2048 queries, the unnormalized attention partial sums

    O_f[q, :] = sum_{k in keys_f, k <= q} exp(s_qk) * V[k, :]
    l_f[q]    = sum_{k in keys_f, k <= q} exp(s_qk)

The host combines the two folds: O = (O_0 + O_1) / (l_0 + l_1).

Causality at block granularity: for query chunk qc (256 queries), only
local key blocks i <= qc are causally live (global kb = 2i+f <= 2qc+1),
and only the LAST one (i == qc) intersects the mask diagonal, so only
that block needs the mask-bias add. The stride-2 interleave makes this
structure identical on every core -> one SPMD program, balanced load.

Per-core phases (fp32r matmuls except P/V in bf16):
  A1: Q^T = Wq @ x^T   for all queries (wq resident, x streamed, mc-outer
      accumulation so the first matmul needs only 640 KB of DMA)
  A2: K^T = Wk @ x_f^T own keys (kc-outer; wk/xkv streamed + prestaged)
  A3: V   = x_f @ Wv^T own keys (wv streamed in dv halves)
  B:  flash loop over (qc, i<=qc): S^T = K @ Q^T into PSUM, mask bias
      added in-place on the diagonal block only, exp on ACT (no max
      subtraction; scores ~N(0,1)), P bf16, O += P^T V and l += column
      sums of P in PSUM; unnormalized O (bf16) and l DMA'd out.

DMA issue order is hand-scheduled (prestage lists) so the PE never
waits on the single hardware DGE queue once A1 starts.
"""

import sys

sys.path.insert(0, "/opt/trn_rl_repo")

import numpy as np

import concourse.bass as bass
import concourse.mybir as mybir
from concourse import tile
from concourse.bass_utils import run_bass_kernel_spmd

F32 = mybir.dt.float32
F32R = mybir.dt.float32r
BF16 = mybir.dt.bfloat16
AF = mybir.ActivationFunctionType

B, S, D = 4, 2048, 1024
NK = 1024          # keys per core (8 local blocks of 128)
NLB = 8            # local key blocks per core
NQC = 8            # query chunks (256 queries each), all queries
QW = 256           # query width of one score tile
NMC = 8            # d_model chunks of 128 (contraction)
NDC = 8            # d_k chunks of 128
NEG = -1.0e6
SCALE = 1.0 / 32.0  # 1/sqrt(D_K)


def _build_nc(reps=1):
    nc = bass.Bass()
    xqT = nc.declare_dram_parameter("xqT", [D, S], F32R, isOutput=False)
    xkvT = nc.declare_dram_parameter("xkvT", [D, NK], F32R, isOutput=False)
    wqT = nc.declare_dram_parameter("wqT", [D, D], F32R, isOutput=False)
    wkT = nc.declare_dram_parameter("wkT", [D, D], F32R, isOutput=False)
    wvT = nc.declare_dram_parameter("wvT", [D, D], F32R, isOutput=False)
    biasT = nc.declare_dram_parameter("biasT", [NQC, 128, QW], F32, isOutput=False)
    out = nc.declare_dram_parameter("out", [S, D], BF16, isOutput=True)
    lT = nc.declare_dram_parameter("lT", [1, S], F32, isOutput=True)

    with tile.TileContext(nc) as tc:
        for _rep in range(reps):
            _build_rep(nc, tc, xqT, xkvT, wqT, wkT, wvT, biasT, out, lT)
    _elide_transitive_waits(nc)
    return nc


class _DrainAlternator:
    """Alternate PSUM->SBUF drains between DVE and ACT to halve the
    serial drain dwell at tile boundaries."""

    def __init__(self, nc):
        self.nc = nc
        self.flip = False

    def __call__(self, dst, src):
        self.flip = not self.flip
        if self.flip:
            self.nc.vector.tensor_copy(dst, src)
        else:
            self.nc.scalar.activation(dst, src, AF.Copy)


def _build_rep(nc, tc, xqT, xkvT, wqT, wkT, wvT, biasT, out, lT):
    drain = _DrainAlternator(nc)
    with tc.tile_pool(name="res", bufs=1) as res:
        # Residents for phase B, per-block so late A-phase drains don't
        # gate early B reads: Q^T [p=dk, dc*S + q]; K^T block i
        # [p=dk-in-dc, dc*128 + k]; V block i [p=k, dv] (bf16).
        qt_sb = res.tile([128, NDC * S], F32R, name="qt_sb")
        kt_sb = res.tile([128, NDC * NK], F32R, name="kt_sb")
        v = [res.tile([128, D], BF16, name=f"v{i}") for i in range(NLB)]
        ones = res.tile([128, 1], BF16, name="ones")
        nc.vector.memset(ones[:], 1.0)

        # "pre" holds tiles prestaged during A1 that must survive into
        # A2/A3 (pool reservations are open->close, so this pool carries
        # only the 24 KB/partition prestage set).
        with tc.tile_pool(name="pre", bufs=1) as pre:
            # staged-load state: xkv halves [h][mc], wk [mc]
            xkv_h = [[None] * NMC for _ in range(2)]
            wk_mc = [None] * NMC

            def load_xkv(pool, h, mc):
                t = pool.tile([128, 512], F32R, name=f"xkv{h}_{mc}",
                              tag=f"xkv{h}_{mc}")
                nc.sync.dma_start(
                    t[:], xkvT[mc * 128:(mc + 1) * 128, h * 512:(h + 1) * 512]
                )
                xkv_h[h][mc] = t

            def load_wk(pool, mc):
                t = pool.tile([128, D], F32R, name=f"wk{mc}", tag=f"wk{mc}")
                nc.sync.dma_start(t[:], wkT[mc * 128:(mc + 1) * 128, :])
                wk_mc[mc] = t

            # ---------------- A1: Q^T = Wq @ x^T (all queries) ----------
            # wq in dc-halves and a hand-interleaved DMA issue order so
            # the first matmul fires ~2 us in and the q2=0 pass chases
            # the DMA stream with minimal lag.
            with (
                tc.tile_pool(name="wqp", bufs=1) as wqp,
                tc.tile_pool(name="xcp", bufs=1) as xcp,
                tc.tile_pool(name="psa", bufs=1, space="PSUM") as psa,
            ):
                XW = 512
                wq_h = [[None, None] for _ in range(NMC)]

                def load_wq_half(mc, h):
                    t = wqp.tile([128, 512], F32R, name=f"wq{mc}_{h}",
                                 tag=f"wq{mc}_{h}")
                    nc.sync.dma_start(
                        t[:], wqT[mc * 128:(mc + 1) * 128, h * 512:(h + 1) * 512]
                    )
                    wq_h[mc][h] = t

                def load_xc(q2, mc):
                    t = xcp.tile([128, XW], F32R, name="xc", tag="xc", bufs=10)
                    nc.sync.dma_start(
                        t[:], xqT[mc * 128:(mc + 1) * 128, q2 * XW:(q2 + 1) * XW]
                    )
                    return t

                # first pass (dc 0-3) needs only the h=0 wq halves: issue
                # them interleaved with q2=0's x chunks, h=1 halves after
                xc0 = []
                for mc in range(NMC):
                    load_wq_half(mc, 0)
                    xc0.append(load_xc(0, mc))
                for mc in range(NMC):
                    load_wq_half(mc, 1)
                for q2 in range(S // XW):
                    xc = xc0 if q2 == 0 else [load_xc(q2, mc) for mc in range(NMC)]
                    # prestage A2 inputs while A1 computes (after this q2's
                    # x chunks so they don't delay the critical DMA stream)
                    if q2 == 1:
                        for mc in range(4):
                            load_xkv(pre, 0, mc)
                    elif q2 == 2:
                        for mc in range(4, NMC):
                            load_xkv(pre, 0, mc)
                    elif q2 == 3:
                        load_wk(pre, 0)
                        load_wk(pre, 1)
                    # two dc-half passes ping-ponging two 4-bank PSUM sets:
                    # the reused set was drained a full pass ago -> no WAR
                    for h in range(2):
                        ps = [
                            psa.tile([128, XW], F32, name=f"psq{j}",
                                     tag=f"ps{h * 4 + j}")
                            for j in range(4)
                        ]
                        for mc in range(NMC):
                            for j in range(4):
                                nc.tensor.matmul(
                                    ps[j][:],
                                    lhsT=wq_h[mc][h][:, j * 128:(j + 1) * 128],
                                    rhs=xc[mc][:],
                                    start=(mc == 0),
                                    stop=(mc == NMC - 1),
                                )
                        for j in range(4):
                            dc = h * 4 + j
                            drain(
                                qt_sb[:, dc * S + q2 * XW: dc * S + (q2 + 1) * XW],
                                ps[j][:],
                            )

            with tc.tile_pool(name="wvp", bufs=1) as wvp:

                def load_wv_half(dvc):
                    h = []
                    for mc in range(NMC):
                        t = wvp.tile([128, 512], F32R, name=f"wv{mc}",
                                     tag=f"wv{mc}")
                        nc.sync.dma_start(
                            t[:],
                            wvT[mc * 128:(mc + 1) * 128, dvc * 512:(dvc + 1) * 512],
                        )
                        h.append(t)
                    return h

                # ------------- A2: K^T = Wk @ x_f^T (own keys) -----------
                with tc.tile_pool(name="xk2", bufs=1) as xk2:
                    with (
                        tc.tile_pool(name="wk2", bufs=1) as wk2,
                        tc.tile_pool(name="psa2", bufs=1, space="PSUM") as psa,
                    ):
                        for mc in range(2, NMC):
                            load_wk(wk2, mc)
                        for mc in range(NMC):
                            load_xkv(xk2, 1, mc)
                        wv_h0 = load_wv_half(0)
                        # kc-outer (key halves); dc-half passes ping-pong
                        # two 4-bank PSUM sets to kill WAR-on-drain stalls
                        for kc in range(2):
                            for dcg in range(2):
                                ps = [
                                    psa.tile([128, 512], F32, name=f"psk{j}",
                                             tag=f"ps{dcg * 4 + j}")
                                    for j in range(4)
                                ]
                                for mc in range(NMC):
                                    for j in range(4):
                                        dc = dcg * 4 + j
                                        nc.tensor.matmul(
                                            ps[j][:],
                                            lhsT=wk_mc[mc][:, dc * 128:(dc + 1) * 128],
                                            rhs=xkv_h[kc][mc][:],
                                            start=(mc == 0),
                                            stop=(mc == NMC - 1),
                                        )
                                for j in range(4):
                                    dc = dcg * 4 + j
                                    drain(
                                        kt_sb[:, dc * NK + kc * 512: dc * NK + (kc + 1) * 512],
                                        ps[j][:],
                                    )

                    # ---------- A3: V = x_f @ Wv^T (own keys, bf16) ------
                    # (inside xk2 scope: A3 reads both xkv halves)
                    with tc.tile_pool(name="psa3", bufs=1, space="PSUM") as psa3:
                        wv = [wv_h0, None]
                        for dvc in range(2):
                            if dvc == 0:
                                wv[1] = load_wv_half(1)
                            for kbg in range(2):
                                pp = (dvc * 2 + kbg) % 2
                                ps = [
                                    psa3.tile([128, 512], F32, name=f"psv{j}",
                                              tag=f"psv{pp * 4 + j}")
                                    for j in range(4)
                                ]
                                for mc in range(NMC):
                                    for kbi in range(4):
                                        nc.tensor.matmul(
                                            ps[kbi][:],
                                            lhsT=xkv_h[kbg][mc][:, kbi * 128:(kbi + 1) * 128],
                                            rhs=wv[dvc][mc][:],
                                            start=(mc == 0),
                                            stop=(mc == NMC - 1),
                                        )
                                for kbi in range(4):
                                    kb = kbg * 4 + kbi
                                    drain(
                                        v[kb][:, dvc * 512:(dvc + 1) * 512],
                                        ps[kbi][:],
                                    )

        # ---------------- Phase B: causal flash attention ----------------
        with (
            tc.tile_pool(name="btp", bufs=1) as btp,
            tc.tile_pool(name="pap", bufs=3) as pap,
            tc.tile_pool(name="otp", bufs=4) as otp,
            tc.tile_pool(name="lap", bufs=1) as lap,
            tc.tile_pool(name="psb", bufs=1, space="PSUM") as psb,
        ):
            bt = []
            for qc in range(NQC):
                t = btp.tile([128, QW], F32, name=f"bt{qc}", tag=f"bt{qc}")
                nc.sync.dma_start(t[:], biasT[qc])
                bt.append(t)
            l_acc = lap.tile([1, S], F32, name="l_acc")
            DV = 512
            NDV = D // DV
            for qc in range(NQC):
                o_ps = [
                    psb.tile([128, DV], F32, name=f"o_ps{j}", tag=f"o{j}")
                    for j in range(2 * NDV)
                ]
                l_ps = psb.tile([1, QW], F32, name="l_ps", tag="l")

                def do_scores(i, qc=qc):
                    st = psb.tile([128, QW], F32, name="st", tag="st", bufs=3)
                    for dc in range(NDC):
                        nc.tensor.matmul(
                            st[:],
                            lhsT=kt_sb[:, dc * NK + i * 128: dc * NK + i * 128 + 128],
                            rhs=qt_sb[:, dc * S + qc * QW: dc * S + qc * QW + QW],
                            start=(dc == 0),
                            stop=(dc == NDC - 1),
                        )
                    if i == qc:
                        # mask bias only intersects the diagonal block;
                        # add it in place on the PSUM scores
                        nc.vector.tensor_add(st[:], st[:], bt[qc][:])
                    pa = pap.tile([128, QW], BF16, name="pa", tag="pa")
                    nc.scalar.activation(pa[:], st[:], AF.Exp, scale=SCALE)
                    return pa

                def do_lo(i, pa, qc=qc):
                    nc.tensor.matmul(
                        l_ps[:],
                        lhsT=ones[:],
                        rhs=pa[:],
                        start=(i == 0),
                        stop=(i == qc),
                    )
                    for qb in range(2):
                        for dvc in range(NDV):
                            nc.tensor.matmul(
                                o_ps[qb * NDV + dvc][:],
                                lhsT=pa[:, qb * 128:(qb + 1) * 128],
                                rhs=v[i][:, dvc * DV:(dvc + 1) * DV],
                                start=(i == 0),
                                stop=(i == qc),
                            )

                # software-pipeline: O/l matmuls trail the score blocks by
                # up to two, so exp latency and the previous chunk's PSUM
                # drains hide behind whole score blocks.
                lag = min(2, qc)
                pas = []
                for i in range(qc + 1):
                    pas.append(do_scores(i))
                    if i >= lag:
                        do_lo(i - lag, pas[i - lag])
                for i in range(qc + 1 - lag, qc + 1):
                    do_lo(i, pas[i])

                nc.vector.tensor_copy(l_acc[:, qc * QW:(qc + 1) * QW], l_ps[:])
                last = qc == NQC - 1
                for qb in range(2):
                    ot = otp.tile([128, D], BF16, name="ot", tag="ot")
                    for dvc in range(NDV):
                        # on the last chunk ACT is free (no exps follow);
                        # alternate the drains to halve the tail dwell
                        if last:
                            drain(ot[:, dvc * DV:(dvc + 1) * DV],
                                  o_ps[qb * NDV + dvc][:])
                        else:
                            nc.vector.tensor_copy(
                                ot[:, dvc * DV:(dvc + 1) * DV],
                                o_ps[qb * NDV + dvc][:],
                            )
                    nc.sync.dma_start(
                        out[qc * QW + qb * 128: qc * QW + qb * 128 + 128, :],
                        ot[:],
                    )
            nc.sync.dma_start(lT[:], l_acc[:])


def _elide_transitive_waits(nc):
    """Drop semaphore waits already implied transitively.

    Hardware matmul (fused LDWEIGHTS) and DMA instruction encodings accept
    only ONE sync wait.  Tile's wait assignment is per-proc minimal but NOT
    transitive, so phase boundaries emit multi-wait matmuls/DMAs.  This pass
    walks the scheduled program (list order is a valid linearization),
    maintains a transitive vector clock per proc (engines and DMA queues are
    each FIFO), and removes waits that are (a) on the instruction's own proc
    (FIFO completion order), or (b) already implied by an earlier retained
    wait's transitive closure.
    """
    import re
    _proc_re = re.compile(r"^(PE|DVE|ACT|Act|Activation|SP|Pool|POOL|DMAHW\d+|DMASW\d+)_")

    def _is_proc_sem(name):
        return bool(_proc_re.match(name or ""))

    hist = {}      # sem id -> list of (tick, snapshot dict)
    state = {}     # proc key -> dict(sem id -> observed tick)
    tickc = {}     # sem id -> cumulative tick

    def snap_at(sem, t):
        h = hist.get(sem)
        if not h:
            return None
        lo, hi, best = 0, len(h) - 1, None
        while lo <= hi:
            mid = (lo + hi) // 2
            if h[mid][0] <= t:
                best = h[mid][1]
                lo = mid + 1
            else:
                hi = mid - 1
        return best

    splits = []
    for blk in nc.m.functions[0].blocks:
        for idx, i in enumerate(blk.instructions):
            si = i.sync_info
            if si is None:
                continue
            ups = [u for u in si.on_update if _is_proc_sem(u.ant_name)]
            own = ups[0].id if ups else ("eng", str(i.engine))
            v = state.setdefault(own, {})
            keep = []
            for w in list(si.on_wait):
                if (
                    w.wait_mode != "sem-ge-imm"
                    or w.wait_reg is not None
                    or not _is_proc_sem(w.ant_name)
                ):
                    keep.append(w)
                    continue
                # Same-proc elision is ONLY safe for PE matmuls: the PE
                # completes matmuls strictly in order (pc-monotone ends), so
                # a PE-self completion wait is redundant.  Other engines have
                # deep pipelines where same-engine WAR/WAW needs the wait.
                pe_self = (
                    w.id == own
                    and type(i).__name__ in ("InstMatmult", "InstLdweights")
                    and w.ant_name.startswith("PE")
                )
                if pe_self or v.get(w.id, 0) >= w.wait_value:
                    continue  # implied: PE FIFO or transitive closure
                keep.append(w)
                v[w.id] = max(v.get(w.id, 0), w.wait_value)
                s = snap_at(w.id, w.wait_value)
                if s:
                    for k2, t2 in s.items():
                        if v.get(k2, 0) < t2:
                            v[k2] = t2
            if len(keep) > 1 and all(_is_proc_sem(w.ant_name) for w in keep):
                # Hardware instruction encodings here accept at most one
                # sync wait: hoist all waits onto standalone sequencer
                # event-semaphore wait ops inserted just before.
                for k, w in enumerate(keep):
                    splits.append(
                        (blk, idx, mybir.InstEventSemaphore(
                            name=f"{i.name}-w{k}",
                            engine=i.engine,
                            sync_info=mybir.SyncInfo(on_wait=[w], on_update=[]),
                        ))
                    )
                keep = []
            if len(keep) != len(si.on_wait):
                si.on_wait = keep
                i.sync_info = si
            for u in ups:
                inc = u.update_value if u.update_mode in ("sem-inc", "sem-add-imm") else 0
                t = tickc.get(u.id, 0) + (inc or 0)
                tickc[u.id] = t
                snapshot = dict(v)
                snapshot[u.id] = t
                hist.setdefault(u.id, []).append((t, snapshot))
            nm = type(i).__name__
            if nm in ("InstMatmult", "InstDMACopy", "InstTensorCopy",
                      "InstTensorTensor", "InstActivation", "InstMemset",
                      "InstTensorScalarPtr", "InstReciprocal", "InstLdweights"):
                assert len(i.sync_info.on_wait) <= 1, (
                    i.name, nm,
                    [(w.ant_name, w.wait_value) for w in i.sync_info.on_wait],
                )
    by_blk = {}
    for blk, idx, inst in splits:
        by_blk.setdefault(id(blk), (blk, []))[1].append((idx, inst))
    for blk, items in by_blk.values():
        for idx, inst in sorted(items, key=lambda t: -t[0]):
            nc.register_instruction(inst)
            blk.instructions.insert(idx, inst)


_CACHE = {}


def _get_nc(reps=1):
    if reps not in _CACHE:
        _CACHE[reps] = _build_nc(reps)
    return _CACHE[reps]


def _check_block_causal(mask):
    """The kernel skips blocks above / keeps blocks below the block
    diagonal without reading the mask there; verify that structure."""
    v = np.asarray(mask).reshape(B, 16, 128, 16, 128)
    blk_all = v.all(axis=(2, 4))
    blk_any = v.any(axis=(2, 4))
    qb = np.arange(16)[:, None]
    kb = np.arange(16)[None, :]
    below = kb < qb
    above = kb > qb
    assert blk_all[:, below].all(), "mask not block-causal (holes below diagonal)"
    assert not blk_any[:, above].any(), "mask not block-causal (nonzero above diagonal)"


def make_in_maps(x, mask, Wq, Wk, Wv):
    x = np.asarray(x, dtype=np.float32)
    mask = np.asarray(mask)
    _check_block_causal(mask)
    wqT = np.ascontiguousarray(np.asarray(Wq, np.float32).T)
    wkT = np.ascontiguousarray(np.asarray(Wk, np.float32).T)
    wvT = np.ascontiguousarray(np.asarray(Wv, np.float32).T)
    in_maps = []
    for c in range(8):
        b, f = divmod(c, 2)
        if f == 0:
            xT = np.ascontiguousarray(x[b].T)  # [D, S]
        else:
            xT = in_maps[-1]["xqT"]  # share with fold 0 of same batch
        xkvT = np.ascontiguousarray(
            xT.reshape(D, 16, 128)[:, f::2, :].reshape(D, NK)
        )
        bias = np.empty((NQC, 128, QW), np.float32)
        for qc in range(NQC):
            kb = 2 * qc + f
            mb = mask[b, qc * QW:(qc + 1) * QW, kb * 128:(kb + 1) * 128]
            bias[qc] = np.where(mb.T, np.float32(0.0), np.float32(NEG))
        in_maps.append(
            dict(
                xqT=xT,
                xkvT=xkvT,
                wqT=wqT,
                wkT=wkT,
                wvT=wvT,
                biasT=np.ascontiguousarray(bias),
            )
        )
    return in_maps


def assemble(results):
    out = np.empty((B, S, D), np.float32)
    for b in range(B):
        o = results[2 * b]["out"].astype(np.float32) + results[2 * b + 1][
            "out"
        ].astype(np.float32)
        l = (
            results[2 * b]["lT"].reshape(S).astype(np.float32)
            + results[2 * b + 1]["lT"].reshape(S).astype(np.float32)
        )
        l = np.where(l == 0.0, 1.0, l)
        out[b] = o / l[:, None]
    return out


def assemble_partial(results, inputs):
    """Assemble from a subset of per-core results; NaN where unknown.
    Requires both folds of a batch to produce that batch's rows."""
    out = np.full((B, S, D), np.nan, np.float32)
    for b in range(B):
        ra, rb = results[2 * b], results[2 * b + 1]
        if ra is None or rb is None:
            continue
        o = ra["out"].astype(np.float32) + rb["out"].astype(np.float32)
        l = ra["lT"].reshape(S).astype(np.float32) + rb["lT"].reshape(S).astype(
            np.float32
        )
        l = np.where(l == 0.0, 1.0, l)
        out[b] = o / l[:, None]
    return out


def kernel(x, mask, Wq, Wk, Wv):
    nc = _get_nc()
    in_maps = make_in_maps(x, mask, Wq, Wk, Wv)
    res = run_bass_kernel_spmd(nc, in_maps, list(range(8)))
    return assemble(res.results)
